# revision 23
# baseline (speedup 1.0000x reference)
"""Trainium2 Bass kernel for nn_Pointnet_Features (PointNet-style stages with Mamba blocks).

Strategy: data-parallel over batch (B=8 -> 8 NeuronCores, one sample per core).
Per-core layout: activations [channels, N] with channels on partitions.
 - stage linear + folded BN + ReLU on PE/ACT
 - mamba in_proj / B,C projections / out_proj as bf16 matmuls on PE
 - depthwise causal conv as shifted scalar_tensor_tensor ops on DVE
 - selective scan via hardware tensor_tensor_scan (h = A*h + delta*B) per
   (state, d-block), chained over 512-col chunks via initial=h[:, -1:];
   A broadcast along the free dim with a step-0 AP
 - LayerNorm via ones-matmul partition reductions (two-pass variance to match
   fp32 overflow/NaN semantics of the reference)
 - final max over N with a NaN-forcing fixup (max + 0*sum)
Stage 3 is processed in column chunks (scan state / conv halo carried across)
to fit SBUF.
"""

import sys
import numpy as np

for _p in ("/opt/trn_rl_repo",):
    if _p not in sys.path:
        sys.path.insert(0, _p)

import ml_dtypes  # noqa: E402

BF16 = ml_dtypes.bfloat16

B_, N_, DIMK = 8, 2048, 1024
DSTATE, DCONV = 16, 4
EPS = 1e-5
NCORES = 8
DIMS = [(3, 64), (64, 128), (128, DIMK)]   # (cin, dim); d_inner = 2*dim
NCHUNKS = [1, 1, 4]                        # column chunks per stage
NF = 512                                   # matmul free tile = one fp32 PSUM bank

DEBUG_STAGE_OUTS = False


def _parts(n):
    return [(i, min(128, n - i)) for i in range(0, n, 128)]


def prep_params(params):
    """Host-side folding of the (fixed) parameters into kernel-ready arrays."""
    t = {}
    for i, p in enumerate(params):
        m = p["mamba"]
        w = np.asarray(p["w"], np.float32)
        bn_scale = np.asarray(p["bn_g"], np.float32) / np.sqrt(
            np.asarray(p["bn_v"], np.float32) + EPS
        )
        W = w * bn_scale[:, None]
        bvec = (
            np.asarray(p["b"], np.float32) - np.asarray(p["bn_m"], np.float32)
        ) * bn_scale + np.asarray(p["bn_b"], np.float32)
        t[f"s{i}_Wt"] = np.ascontiguousarray(W.T).astype(BF16)            # [c, m]
        t[f"s{i}_b"] = bvec[:, None].astype(np.float32)
        t[f"s{i}_inwT"] = np.ascontiguousarray(
            np.asarray(m["in_w"], np.float32).T
        ).astype(BF16)                                                     # [m, 2di]
        t[f"s{i}_inb"] = np.asarray(m["in_b"], np.float32)[:, None].astype(np.float32)
        t[f"s{i}_cw"] = np.ascontiguousarray(
            np.asarray(m["cw"], np.float32)[:, 0, :]
        ).astype(np.float32)                                               # [di, 4]
        t[f"s{i}_cb"] = np.asarray(m["cb"], np.float32)[:, None].astype(np.float32)
        t[f"s{i}_A"] = (-np.exp(np.asarray(m["A"], np.float32))).astype(np.float32)
        t[f"s{i}_D"] = np.asarray(m["D"], np.float32)[:, None].astype(np.float32)
        bc = np.concatenate(
            [np.asarray(m["Bw"], np.float32).T, np.asarray(m["Cw"], np.float32).T],
            axis=1,
        )
        t[f"s{i}_bcwT"] = np.ascontiguousarray(bc).astype(BF16)            # [di, 32]
        t[f"s{i}_owT"] = np.ascontiguousarray(
            np.asarray(m["ow"], np.float32).T
        ).astype(BF16)                                                     # [di, dim]
        t[f"s{i}_ob"] = np.asarray(m["ob"], np.float32)[:, None].astype(np.float32)
        t[f"s{i}_lng"] = np.asarray(m["ln_g"], np.float32)[:, None].astype(np.float32)
        t[f"s{i}_lnb"] = np.asarray(m["ln_b"], np.float32)[:, None].astype(np.float32)
    t["ones_bf"] = np.ones((1, 128), BF16)
    t["ones_f32"] = np.ones((1, 128), np.float32)
    t["ones_col_bf"] = np.ones((128, 1), BF16)
    return t


def build_program(param_arrays, compile=True):
    import concourse.bass as bass
    import concourse.bacc as bacc
    import concourse.tile as tile
    from concourse import mybir

    f32 = mybir.dt.float32
    bf16 = mybir.dt.bfloat16
    Alu = mybir.AluOpType
    Act = mybir.ActivationFunctionType

    nc = bacc.Bacc("TRN2", debug=False)

    dram = {}
    for name, arr in param_arrays.items():
        dt = bf16 if arr.dtype == BF16 else mybir.dt.from_np(arr.dtype)
        dram[name] = nc.dram_tensor(name, list(arr.shape), dt, kind="ExternalInput")
    dram["xin"] = nc.dram_tensor("xin", [3, N_], f32, kind="ExternalInput")
    dram["out"] = nc.dram_tensor("out", [DIMK, 1], f32, kind="ExternalOutput")
    dbg = {}
    if DEBUG_STAGE_OUTS:
        for i, (_, m) in enumerate(DIMS):
            dbg[i] = nc.dram_tensor(f"dbg_o{i}", [m, N_], f32, kind="ExternalOutput")

    def bcast_free(col_ap, n):
        """[P,1] AP -> [P,n] AP broadcasting along the free dim (step 0)."""
        return bass.AP(
            tensor=col_ap.tensor,
            offset=col_ap.offset,
            ap=[col_ap.ap[0], [0, n]],
        )

    with tile.TileContext(nc) as tc:
        with (
            tc.tile_pool(name="acts", bufs=1) as acts,
            tc.tile_pool(name="consts", bufs=1) as consts,
        ):
            ones_bf = consts.tile([1, 128], bf16, tag="ones_bf")
            nc.sync.dma_start(out=ones_bf, in_=dram["ones_bf"].ap())
            ones_f32 = consts.tile([1, 128], f32, tag="ones_f32")
            nc.sync.dma_start(out=ones_f32, in_=dram["ones_f32"].ap())
            ones_col = consts.tile([128, 1], bf16, tag="ones_col")
            nc.sync.dma_start(out=ones_col, in_=dram["ones_col_bf"].ap())
            big_t = consts.tile([1, 1], f32, tag="big_const")
            nc.vector.memset(big_t, 2.0 ** 110)

            x0 = acts.tile([3, N_], bf16, tag="x_in")
            x_tiles = [x0]

            for si, (c, m) in enumerate(DIMS):
                di = 2 * m
                c_p, m_p, d_p = _parts(c), _parts(m), _parts(di)
                last = si == len(DIMS) - 1
                nch = NCHUNKS[si]
                CN = N_ // nch            # columns per chunk
                NFC = CN // NF            # 512-col tiles per chunk

                with (
                    tc.tile_pool(name=f"s{si}_res", bufs=1) as res,
                    tc.tile_pool(name=f"s{si}_w", bufs=2) as wpool,
                    tc.tile_pool(name=f"s{si}_tmp", bufs=2) as tmp,
                    tc.tile_pool(name=f"s{si}_mm", bufs=3, space="PSUM") as psmm,
                    tc.tile_pool(name=f"s{si}_ps1", bufs=1, space="PSUM") as ps1,
                ):
                    if si == 0:
                        x_f32 = tmp.tile([3, N_], f32, tag="x_in_f32", bufs=1)
                        nc.sync.dma_start(out=x_f32, in_=dram["xin"].ap())
                        nc.vector.tensor_copy(out=x0, in_=x_f32)

                    def vec(name, plist, tag):
                        ts = []
                        for (st, sz) in plist:
                            v = res.tile([sz, 1], f32, tag=f"{tag}{st}", name=f"{tag}{st}")
                            nc.sync.dma_start(
                                out=v, in_=dram[name].ap()[st : st + sz, :]
                            )
                            ts.append(v)
                        return ts

                    b_v = vec(f"s{si}_b", m_p, "b")
                    inb_v = vec(f"s{si}_inb", _parts(2 * di), "inb")
                    cb_v = vec(f"s{si}_cb", d_p, "cb")
                    D_v = vec(f"s{si}_D", d_p, "D")
                    ob_v = vec(f"s{si}_ob", m_p, "ob")
                    lng_v = vec(f"s{si}_lng", m_p, "lng")
                    lnb_v = vec(f"s{si}_lnb", m_p, "lnb")

                    A_t, cw_t, bcw_t = [], [], []
                    for bi, (st, sz) in enumerate(d_p):
                        a = res.tile([sz, DSTATE], f32, tag=f"A{bi}", name=f"A{bi}")
                        nc.sync.dma_start(
                            out=a, in_=dram[f"s{si}_A"].ap()[st : st + sz, :]
                        )
                        A_t.append(a)
                        cwt = res.tile([sz, DCONV], f32, tag=f"cw{bi}", name=f"cw{bi}")
                        nc.sync.dma_start(
                            out=cwt, in_=dram[f"s{si}_cw"].ap()[st : st + sz, :]
                        )
                        cw_t.append(cwt)
                        w = res.tile(
                            [sz, 2 * DSTATE], bf16, tag=f"bcw{bi}", name=f"bcw{bi}"
                        )
                        nc.sync.dma_start(
                            out=w, in_=dram[f"s{si}_bcwT"].ap()[st : st + sz, :]
                        )
                        bcw_t.append(w)

                    Wt_t = []
                    for ci, (cst, csz) in enumerate(c_p):
                        w = res.tile([csz, m], bf16, tag=f"Wt{ci}", name=f"Wt{ci}")
                        nc.sync.dma_start(
                            out=w, in_=dram[f"s{si}_Wt"].ap()[cst : cst + csz, :]
                        )
                        Wt_t.append(w)

                    # cross-chunk state
                    halo_t, hcar_t, mxa_t, sma_t = [], [], [], []
                    for bi, (st, sz) in enumerate(d_p):
                        ht = res.tile(
                            [sz, DCONV - 1], f32, tag=f"halo{bi}", name=f"halo{bi}"
                        )
                        nc.vector.memset(ht, 0.0)
                        halo_t.append(ht)
                        hc = res.tile(
                            [sz, DSTATE], f32, tag=f"hcar{bi}", name=f"hcar{bi}"
                        )
                        nc.vector.memset(hc, 0.0)
                        hcar_t.append(hc)
                    if last:
                        for mi, (mst, msz) in enumerate(m_p):
                            mx = res.tile([msz, 1], f32, tag=f"mxa{mi}", name=f"mxa{mi}")
                            mxa_t.append(mx)
                            sm = res.tile([msz, 1], f32, tag=f"sma{mi}", name=f"sma{mi}")
                            sma_t.append(sm)

                    # stage output (full width, persists into next stage)
                    o_t = []
                    if not last:
                        for mi, (mst, msz) in enumerate(m_p):
                            o = acts.tile(
                                [msz, N_], bf16, tag=f"o{si}_{mi}", name=f"o{si}_{mi}"
                            )
                            o_t.append(o)

                    for ch in range(nch):
                        cb = ch * CN

                        def gsl(f):  # global column slice of 512-tile f
                            return slice(cb + f * NF, cb + (f + 1) * NF)

                        def lsl(f):  # chunk-local slice
                            return slice(f * NF, (f + 1) * NF)

                        # ---- linear + BN + relu ----
                        r_t = []
                        for mi, (mst, msz) in enumerate(m_p):
                            r = res.tile([msz, CN], bf16, tag=f"r{mi}", name=f"r{mi}", bufs=(2 if nch > 1 else 1))
                            r_t.append(r)
                            for f in range(NFC):
                                pl = psmm.tile([msz, NF], f32, tag="mm")
                                for ci, (cst, csz) in enumerate(c_p):
                                    nc.tensor.matmul(
                                        out=pl,
                                        lhsT=Wt_t[ci][:, mst : mst + msz],
                                        rhs=x_tiles[ci][:, gsl(f)],
                                        start=(ci == 0),
                                        stop=(ci == len(c_p) - 1),
                                    )
                                nc.scalar.activation(
                                    out=r[:, lsl(f)],
                                    in_=pl,
                                    func=Act.Relu,
                                    bias=b_v[mi],
                                    scale=1.0,
                                )

                        # ---- in_proj, conv, silu, D-init, B/C ----
                        delta_t, y_t = [], []
                        bc_ps = ps1.tile([2 * DSTATE, CN], f32, tag="big")
                        for bi, (dst, dsz) in enumerate(d_p):
                            xcp = tmp.tile(
                                [dsz, DCONV - 1 + CN], bf16, tag="xcpad"
                            )
                            nc.vector.tensor_copy(
                                out=xcp[:, : DCONV - 1], in_=halo_t[bi]
                            )
                            for f in range(NFC):
                                pxc = psmm.tile([dsz, NF], f32, tag="mm")
                                for mi, (mst, msz) in enumerate(m_p):
                                    wl = wpool.tile(
                                        [msz, dsz], bf16, tag=f"wxc{mi}", name="wl"
                                    )
                                    nc.sync.dma_start(
                                        out=wl,
                                        in_=dram[f"s{si}_inwT"].ap()[
                                            mst : mst + msz, dst : dst + dsz
                                        ],
                                    )
                                    nc.tensor.matmul(
                                        out=pxc,
                                        lhsT=wl,
                                        rhs=r_t[mi][:, lsl(f)],
                                        start=(mi == 0),
                                        stop=(mi == len(m_p) - 1),
                                    )
                                nc.scalar.activation(
                                    out=xcp[
                                        :,
                                        DCONV - 1 + f * NF : DCONV - 1 + (f + 1) * NF,
                                    ],
                                    in_=pxc,
                                    func=Act.Identity,
                                    bias=inb_v[dst // 128],
                                    scale=1.0,
                                )
                            # save halo for next chunk (last 3 columns)
                            if ch + 1 < nch:
                                nc.vector.tensor_copy(
                                    out=halo_t[bi], in_=xcp[:, CN : CN + DCONV - 1]
                                )
                            dlt = res.tile(
                                [dsz, CN], bf16, tag=f"delta{bi}", name=f"delta{bi}",
                                bufs=(2 if nch > 1 else 1),
                            )
                            delta_t.append(dlt)
                            zst = di + dst
                            for f in range(NFC):
                                pz = psmm.tile([dsz, NF], f32, tag="mm")
                                for mi, (mst, msz) in enumerate(m_p):
                                    wl = wpool.tile(
                                        [msz, dsz], bf16, tag=f"wz{mi}", name="wl"
                                    )
                                    nc.sync.dma_start(
                                        out=wl,
                                        in_=dram[f"s{si}_inwT"].ap()[
                                            mst : mst + msz, zst : zst + dsz
                                        ],
                                    )
                                    nc.tensor.matmul(
                                        out=pz,
                                        lhsT=wl,
                                        rhs=r_t[mi][:, lsl(f)],
                                        start=(mi == 0),
                                        stop=(mi == len(m_p) - 1),
                                    )
                                nc.scalar.activation(
                                    out=dlt[:, lsl(f)],
                                    in_=pz,
                                    func=Act.Sigmoid,
                                    bias=inb_v[zst // 128],
                                    scale=1.0,
                                )
                            acc = tmp.tile([dsz, CN], f32, tag="convacc", bufs=2)
                            nc.vector.tensor_scalar_mul(
                                out=acc, in0=xcp[:, 0:CN], scalar1=cw_t[bi][:, 0:1]
                            )
                            for k in range(1, DCONV):
                                nc.vector.scalar_tensor_tensor(
                                    out=acc,
                                    in0=xcp[:, k : k + CN],
                                    scalar=cw_t[bi][:, k : k + 1],
                                    in1=acc,
                                    op0=Alu.mult,
                                    op1=Alu.add,
                                )
                            nc.vector.tensor_scalar_add(
                                out=acc, in0=acc, scalar1=cb_v[bi]
                            )
                            sg = tmp.tile([dsz, CN], bf16, tag="sg", bufs=2)
                            nc.scalar.activation(
                                out=sg, in_=acc, func=Act.Sigmoid, bias=0.0, scale=1.0
                            )
                            xt = tmp.tile([dsz, CN], bf16, tag="xt", bufs=2)
                            nc.vector.tensor_mul(xt, acc, sg)
                            y = res.tile([dsz, CN], bf16, tag=f"y{bi}", name=f"y{bi}", bufs=(2 if nch > 1 else 1))
                            y_t.append(y)
                            nc.vector.tensor_scalar_mul(out=y, in0=xt, scalar1=D_v[bi])
                            for f in range(NFC):
                                nc.tensor.matmul(
                                    out=bc_ps[:, lsl(f)],
                                    lhsT=bcw_t[bi],
                                    rhs=xt[:, lsl(f)],
                                    start=(bi == 0),
                                    stop=(bi == len(d_p) - 1),
                                )
                        bc_sb = res.tile([2 * DSTATE, CN], bf16, tag="bc_sb", bufs=(2 if nch > 1 else 1))
                        nc.scalar.copy(out=bc_sb, in_=bc_ps)

                        # ---- scan ----
                        for s in range(DSTATE):
                            brow = tmp.tile([1, CN], bf16, tag="brow", bufs=2)
                            nc.sync.dma_start(out=brow, in_=bc_sb[s : s + 1, :])
                            crow = tmp.tile([1, CN], bf16, tag="crow", bufs=2)
                            nc.sync.dma_start(
                                out=crow, in_=bc_sb[DSTATE + s : DSTATE + s + 1, :]
                            )
                            brep = tmp.tile([128, CN], bf16, tag="brep", bufs=2)
                            crep = tmp.tile([128, CN], bf16, tag="crep", bufs=2)
                            for f in range(NFC):
                                brep_ps = psmm.tile([128, NF], f32, tag="mm")
                                nc.tensor.matmul(
                                    out=brep_ps,
                                    lhsT=ones_bf,
                                    rhs=brow[:, lsl(f)],
                                    start=True,
                                    stop=True,
                                )
                                nc.scalar.copy(out=brep[:, lsl(f)], in_=brep_ps)
                                crep_ps = psmm.tile([128, NF], f32, tag="mm")
                                nc.tensor.matmul(
                                    out=crep_ps,
                                    lhsT=ones_bf,
                                    rhs=crow[:, lsl(f)],
                                    start=True,
                                    stop=True,
                                )
                                nc.scalar.copy(out=crep[:, lsl(f)], in_=crep_ps)

                            for bi, (dst, dsz) in enumerate(d_p):
                                h = tmp.tile([dsz, NF], f32, tag="h", name="h")
                                for f in range(NFC):
                                    u = tmp.tile([dsz, NF], bf16, tag="u", bufs=4)
                                    nc.vector.tensor_mul(
                                        u, delta_t[bi][:, lsl(f)], brep[:dsz, lsl(f)]
                                    )
                                    init = (
                                        hcar_t[bi][:, s : s + 1]
                                        if f == 0
                                        else h[:, NF - 1 : NF]
                                    )
                                    nc.vector.tensor_tensor_scan(
                                        out=h,
                                        data0=bcast_free(A_t[bi][:, s : s + 1], NF),
                                        data1=u,
                                        initial=init,
                                        op0=Alu.mult,
                                        op1=Alu.add,
                                    )
                                    hc = tmp.tile([dsz, NF], bf16, tag="hc", bufs=4)
                                    nc.vector.tensor_mul(hc, h, crep[:dsz, lsl(f)])
                                    nc.vector.tensor_add(
                                        y_t[bi][:, lsl(f)], y_t[bi][:, lsl(f)], hc
                                    )
                                if ch + 1 < nch:
                                    nc.vector.tensor_copy(
                                        out=hcar_t[bi][:, s : s + 1],
                                        in_=h[:, NF - 1 : NF],
                                    )

                        # ---- out_proj + ob + residual (written in-place into r) ----
                        op_t = r_t
                        for mi, (mst, msz) in enumerate(m_p):
                            for f in range(NFC):
                                po = psmm.tile([msz, NF], f32, tag="mm")
                                for bi, (dst, dsz) in enumerate(d_p):
                                    wl = wpool.tile(
                                        [dsz, msz], bf16, tag=f"wo{bi % 4}", name="wl"
                                    )
                                    nc.sync.dma_start(
                                        out=wl,
                                        in_=dram[f"s{si}_owT"].ap()[
                                            dst : dst + dsz, mst : mst + msz
                                        ],
                                    )
                                    nc.tensor.matmul(
                                        out=po,
                                        lhsT=wl,
                                        rhs=y_t[bi][:, lsl(f)],
                                        start=(bi == 0),
                                        stop=(bi == len(d_p) - 1),
                                    )
                                nc.vector.scalar_tensor_tensor(
                                    out=op_t[mi][:, lsl(f)],
                                    in0=po,
                                    scalar=ob_v[mi],
                                    in1=r_t[mi][:, lsl(f)],
                                    op0=Alu.add,
                                    op1=Alu.add,
                                )

                        # ---- LayerNorm over channels ----
                        mu_ps = ps1.tile([1, CN], f32, tag="big")
                        for mi, (mst, msz) in enumerate(m_p):
                            for f in range(NFC):
                                nc.tensor.matmul(
                                    out=mu_ps[:, lsl(f)],
                                    lhsT=ones_col[:msz, :],
                                    rhs=op_t[mi][:, lsl(f)],
                                    start=(mi == 0),
                                    stop=(mi == len(m_p) - 1),
                                )
                        mu_sb = tmp.tile([1, CN], f32, tag="stat_sb", bufs=4)
                        nc.scalar.mul(out=mu_sb, in_=mu_ps, mul=1.0 / m)
                        for f in range(NFC):
                            murep = psmm.tile([128, NF], f32, tag="mm")
                            nc.tensor.matmul(
                                out=murep,
                                lhsT=ones_f32,
                                rhs=mu_sb[:, lsl(f)],
                                start=True,
                                stop=True,
                            )
                            for mi, (mst, msz) in enumerate(m_p):
                                nc.vector.tensor_sub(
                                    op_t[mi][:, lsl(f)],
                                    op_t[mi][:, lsl(f)],
                                    murep[:msz, :],
                                )
                        var_ps = ps1.tile([1, CN], f32, tag="big")
                        for mi, (mst, msz) in enumerate(m_p):
                            sq = tmp.tile([msz, CN], bf16, tag="sq")
                            nc.scalar.square(out=sq, in_=op_t[mi])
                            for f in range(NFC):
                                nc.tensor.matmul(
                                    out=var_ps[:, lsl(f)],
                                    lhsT=ones_col[:msz, :],
                                    rhs=sq[:, lsl(f)],
                                    start=(mi == 0),
                                    stop=(mi == len(m_p) - 1),
                                )
                        # rstd = 1/sqrt(var/m + eps), computed as
                        # 2^-32 / sqrt((var/m + eps) * 2^-64) so the ACT sqrt
                        # stays in its valid range even for huge variances;
                        # inf clamps to 2^110 (rstd ~ 0) and NaN columns are
                        # scrubbed for the sqrt then re-injected.
                        t_v = tmp.tile([1, CN], f32, tag="stat_sb", bufs=4)
                        nc.vector.tensor_scalar(
                            out=t_v,
                            in0=var_ps,
                            scalar1=(2.0 ** -64) / m,
                            scalar2=EPS * (2.0 ** -64),
                            op0=Alu.mult,
                            op1=Alu.add,
                        )
                        mnan = tmp.tile([1, CN], f32, tag="stat_sb", bufs=4)
                        nc.vector.tensor_tensor(
                            out=mnan, in0=t_v, in1=t_v, op=Alu.not_equal
                        )
                        mnan8 = tmp.tile(
                            [1, CN], mybir.dt.uint8, tag="mnan8", bufs=2
                        )
                        nc.vector.tensor_tensor(
                            out=mnan8, in0=t_v, in1=t_v, op=Alu.not_equal
                        )
                        t_c = tmp.tile([1, CN], f32, tag="stat_sb", bufs=4)
                        nc.vector.tensor_scalar_min(
                            out=t_c, in0=t_v, scalar1=2.0 ** 110
                        )
                        nc.vector.copy_predicated(
                            out=t_c, mask=mnan8, data=bcast_free(big_t, CN)
                        )
                        std_sb = tmp.tile([1, CN], f32, tag="stat_sb", bufs=4)
                        nc.scalar.activation(
                            out=std_sb, in_=t_c, func=Act.Sqrt, bias=0.0, scale=1.0
                        )
                        r1_sb = tmp.tile([1, CN], f32, tag="stat_sb", bufs=4)
                        nc.vector.reciprocal(out=r1_sb, in_=std_sb)
                        # nanadd: NaN where var was NaN, else 0
                        nc.vector.tensor_scalar_mul(
                            out=mnan, in0=mnan, scalar1=3.0e38
                        )
                        nc.vector.tensor_mul(mnan, mnan, mnan)
                        nc.vector.tensor_scalar_mul(out=mnan, in0=mnan, scalar1=0.0)
                        rstd_sb = tmp.tile([1, CN], f32, tag="stat_sb", bufs=4)
                        nc.vector.scalar_tensor_tensor(
                            out=rstd_sb,
                            in0=r1_sb,
                            scalar=2.0 ** -32,
                            in1=mnan,
                            op0=Alu.mult,
                            op1=Alu.add,
                        )

                        for f in range(NFC):
                            rstdrep = psmm.tile([128, NF], f32, tag="mm")
                            nc.tensor.matmul(
                                out=rstdrep,
                                lhsT=ones_f32,
                                rhs=rstd_sb[:, lsl(f)],
                                start=True,
                                stop=True,
                            )
                            for mi, (mst, msz) in enumerate(m_p):
                                if last:
                                    od = tmp.tile(
                                        [msz, NF], bf16, tag="ofin", bufs=3, name="od"
                                    )
                                else:
                                    od = o_t[mi][:, gsl(f)]
                                nc.vector.scalar_tensor_tensor(
                                    out=od,
                                    in0=op_t[mi][:, lsl(f)],
                                    scalar=lng_v[mi],
                                    in1=rstdrep[:msz, :],
                                    op0=Alu.mult,
                                    op1=Alu.mult,
                                )
                                nc.vector.tensor_scalar_add(
                                    out=od, in0=od, scalar1=lnb_v[mi]
                                )
                                if last:
                                    # accumulate max and sum over this 512-col piece
                                    mx = tmp.tile([msz, 1], f32, tag="mx", bufs=3)
                                    nc.vector.tensor_reduce(
                                        out=mx,
                                        in_=od,
                                        axis=mybir.AxisListType.X,
                                        op=Alu.max,
                                    )
                                    sm = tmp.tile([msz, 1], f32, tag="sm", bufs=3)
                                    nc.vector.tensor_reduce(
                                        out=sm,
                                        in_=od,
                                        axis=mybir.AxisListType.X,
                                        op=Alu.add,
                                    )
                                    if ch == 0 and f == 0:
                                        nc.vector.tensor_copy(out=mxa_t[mi], in_=mx)
                                        nc.vector.tensor_copy(out=sma_t[mi], in_=sm)
                                    else:
                                        nc.vector.tensor_max(
                                            mxa_t[mi], mxa_t[mi], mx
                                        )
                                        nc.vector.tensor_add(
                                            sma_t[mi], sma_t[mi], sm
                                        )
                                if DEBUG_STAGE_OUTS:
                                    of = tmp.tile(
                                        [msz, NF], f32, tag="dbg_cp", name="of"
                                    )
                                    nc.vector.tensor_copy(out=of, in_=od)
                                    nc.sync.dma_start(
                                        out=dbg[si].ap()[mst : mst + msz, gsl(f)],
                                        in_=of,
                                    )

                    if last:
                        for mi, (mst, msz) in enumerate(m_p):
                            fx = tmp.tile([msz, 1], f32, tag="fx", bufs=2)
                            nc.vector.tensor_scalar_mul(
                                out=fx, in0=sma_t[mi], scalar1=0.0
                            )
                            rs = tmp.tile([msz, 1], f32, tag="rs", bufs=2)
                            nc.vector.tensor_add(rs, mxa_t[mi], fx)
                            nc.sync.dma_start(
                                out=dram["out"].ap()[mst : mst + msz, :], in_=rs
                            )
                    else:
                        x_tiles = o_t
    if compile:
        nc.compile()
    return nc


_CACHE = {}


def kernel(points, params):
    """points: [8, 2048, 3] fp32; params: list of 3 stage dicts. Returns [8, 1024] fp32."""
    from concourse.bass_utils import run_bass_kernel_spmd

    pts = np.asarray(points, np.float32)
    arrs = prep_params(params)

    if "prog" not in _CACHE:
        _CACHE["prog"] = build_program(arrs)
    nc = _CACHE["prog"]

    in_maps = []
    for i in range(NCORES):
        im = dict(arrs)
        im["xin"] = np.ascontiguousarray(pts[i].T).astype(np.float32)
        in_maps.append(im)

    res = run_bass_kernel_spmd(nc, in_maps, core_ids=list(range(NCORES)))
    out = np.stack([r["out"][:, 0] for r in res.results]).astype(np.float32)
    return out


# revision 28
# speedup vs baseline: 1.2769x; 1.2769x over previous
"""Trainium2 Bass kernel for nn_Pointnet_Features (PointNet-style stages with Mamba blocks).

Strategy: data-parallel over batch (B=8 -> 8 NeuronCores, one sample per core).
Per-core layout: activations [channels, N] with channels on partitions.
 - stage linear + folded BN + ReLU on PE/ACT
 - mamba in_proj / B,C projections / out_proj as bf16 matmuls on PE
 - depthwise causal conv as shifted scalar_tensor_tensor ops on DVE
 - selective scan via hardware tensor_tensor_scan (h = A*h + delta*B) per
   (state, d-block), chained over 512-col chunks via initial=h[:, -1:];
   A broadcast along the free dim with a step-0 AP
 - LayerNorm via ones-matmul partition reductions (two-pass variance to match
   fp32 overflow/NaN semantics of the reference)
 - final max over N with a NaN-forcing fixup (max + 0*sum)
Stage 3 is processed in column chunks (scan state / conv halo carried across)
to fit SBUF.
"""

import sys
import numpy as np

for _p in ("/opt/trn_rl_repo",):
    if _p not in sys.path:
        sys.path.insert(0, _p)

import ml_dtypes  # noqa: E402

BF16 = ml_dtypes.bfloat16

B_, N_, DIMK = 8, 2048, 1024
DSTATE, DCONV = 16, 4
EPS = 1e-5
NCORES = 8
DIMS = [(3, 64), (64, 128), (128, DIMK)]   # (cin, dim); d_inner = 2*dim
NCHUNKS = [1, 1, 4]                        # column chunks per stage
NF = 512                                   # matmul free tile = one fp32 PSUM bank

DEBUG_STAGE_OUTS = False


def _parts(n):
    return [(i, min(128, n - i)) for i in range(0, n, 128)]


def prep_params(params):
    """Host-side folding of the (fixed) parameters into kernel-ready arrays."""
    t = {}
    for i, p in enumerate(params):
        m = p["mamba"]
        w = np.asarray(p["w"], np.float32)
        bn_scale = np.asarray(p["bn_g"], np.float32) / np.sqrt(
            np.asarray(p["bn_v"], np.float32) + EPS
        )
        W = w * bn_scale[:, None]
        bvec = (
            np.asarray(p["b"], np.float32) - np.asarray(p["bn_m"], np.float32)
        ) * bn_scale + np.asarray(p["bn_b"], np.float32)
        t[f"s{i}_Wt"] = np.ascontiguousarray(W.T).astype(BF16)            # [c, m]
        t[f"s{i}_b"] = bvec[:, None].astype(np.float32)
        t[f"s{i}_inwT"] = np.ascontiguousarray(
            np.asarray(m["in_w"], np.float32).T
        ).astype(BF16)                                                     # [m, 2di]
        t[f"s{i}_inb"] = np.asarray(m["in_b"], np.float32)[:, None].astype(np.float32)
        t[f"s{i}_cw"] = np.ascontiguousarray(
            np.asarray(m["cw"], np.float32)[:, 0, :]
        ).astype(np.float32)                                               # [di, 4]
        t[f"s{i}_cb"] = np.asarray(m["cb"], np.float32)[:, None].astype(np.float32)
        t[f"s{i}_A"] = (-np.exp(np.asarray(m["A"], np.float32))).astype(np.float32)
        t[f"s{i}_D"] = np.asarray(m["D"], np.float32)[:, None].astype(np.float32)
        bc = np.concatenate(
            [np.asarray(m["Bw"], np.float32).T, np.asarray(m["Cw"], np.float32).T],
            axis=1,
        )
        t[f"s{i}_bcwT"] = np.ascontiguousarray(bc).astype(BF16)            # [di, 32]
        t[f"s{i}_owT"] = np.ascontiguousarray(
            np.asarray(m["ow"], np.float32).T
        ).astype(BF16)                                                     # [di, dim]
        t[f"s{i}_ob"] = np.asarray(m["ob"], np.float32)[:, None].astype(np.float32)
        t[f"s{i}_lng"] = np.asarray(m["ln_g"], np.float32)[:, None].astype(np.float32)
        t[f"s{i}_lnb"] = np.asarray(m["ln_b"], np.float32)[:, None].astype(np.float32)
    t["ones_bf"] = np.ones((1, 128), BF16)
    t["ones_f32"] = np.ones((1, 128), np.float32)
    t["ones_col_bf"] = np.ones((128, 1), BF16)
    return t


def build_program(param_arrays, compile=True):
    import concourse.bass as bass
    import concourse.bacc as bacc
    import concourse.tile as tile
    from concourse import mybir

    f32 = mybir.dt.float32
    bf16 = mybir.dt.bfloat16
    Alu = mybir.AluOpType
    Act = mybir.ActivationFunctionType

    nc = bacc.Bacc("TRN2", debug=False)

    dram = {}
    for name, arr in param_arrays.items():
        dt = bf16 if arr.dtype == BF16 else mybir.dt.from_np(arr.dtype)
        dram[name] = nc.dram_tensor(name, list(arr.shape), dt, kind="ExternalInput")
    dram["xin"] = nc.dram_tensor("xin", [3, N_], f32, kind="ExternalInput")
    dram["out"] = nc.dram_tensor("out", [DIMK, 1], f32, kind="ExternalOutput")
    dbg = {}
    if DEBUG_STAGE_OUTS:
        for i, (_, m) in enumerate(DIMS):
            dbg[i] = nc.dram_tensor(f"dbg_o{i}", [m, N_], f32, kind="ExternalOutput")

    def bcast_free(col_ap, n):
        """[P,1] AP -> [P,n] AP broadcasting along the free dim (step 0)."""
        return bass.AP(
            tensor=col_ap.tensor,
            offset=col_ap.offset,
            ap=[col_ap.ap[0], [0, n]],
        )

    with tile.TileContext(nc) as tc:
        with (
            tc.tile_pool(name="acts", bufs=1) as acts,
            tc.tile_pool(name="consts", bufs=1) as consts,
        ):
            ones_bf = consts.tile([1, 128], bf16, tag="ones_bf")
            nc.sync.dma_start(out=ones_bf, in_=dram["ones_bf"].ap())
            ones_f32 = consts.tile([1, 128], f32, tag="ones_f32")
            nc.sync.dma_start(out=ones_f32, in_=dram["ones_f32"].ap())
            ones_col = consts.tile([128, 1], bf16, tag="ones_col")
            nc.sync.dma_start(out=ones_col, in_=dram["ones_col_bf"].ap())
            big_t = consts.tile([1, 1], f32, tag="big_const")
            nc.vector.memset(big_t, 2.0 ** 110)

            x0 = acts.tile([3, N_], bf16, tag="x_in")
            x_tiles = [x0]

            for si, (c, m) in enumerate(DIMS):
                di = 2 * m
                c_p, m_p, d_p = _parts(c), _parts(m), _parts(di)
                last = si == len(DIMS) - 1
                nch = NCHUNKS[si]
                CN = N_ // nch            # columns per chunk
                NFC = CN // NF            # 512-col tiles per chunk

                with (
                    tc.tile_pool(name=f"s{si}_res", bufs=1) as res,
                    tc.tile_pool(name=f"s{si}_w", bufs=2) as wpool,
                    tc.tile_pool(name=f"s{si}_tmp", bufs=2) as tmp,
                    tc.tile_pool(name=f"s{si}_mm", bufs=3, space="PSUM") as psmm,
                    tc.tile_pool(name=f"s{si}_ps1", bufs=1, space="PSUM") as ps1,
                ):
                    if si == 0:
                        x_f32 = tmp.tile([3, N_], f32, tag="x_in_f32", bufs=1)
                        nc.sync.dma_start(out=x_f32, in_=dram["xin"].ap())
                        nc.vector.tensor_copy(out=x0, in_=x_f32)

                    def vec(name, plist, tag):
                        ts = []
                        for (st, sz) in plist:
                            v = res.tile([sz, 1], f32, tag=f"{tag}{st}", name=f"{tag}{st}")
                            nc.sync.dma_start(
                                out=v, in_=dram[name].ap()[st : st + sz, :]
                            )
                            ts.append(v)
                        return ts

                    b_v = vec(f"s{si}_b", m_p, "b")
                    inb_v = vec(f"s{si}_inb", _parts(2 * di), "inb")
                    cb_v = vec(f"s{si}_cb", d_p, "cb")
                    D_v = vec(f"s{si}_D", d_p, "D")
                    ob_v = vec(f"s{si}_ob", m_p, "ob")
                    lng_v = vec(f"s{si}_lng", m_p, "lng")
                    lnb_v = vec(f"s{si}_lnb", m_p, "lnb")

                    A_t, cw_t, bcw_t = [], [], []
                    for bi, (st, sz) in enumerate(d_p):
                        a = res.tile([sz, DSTATE], f32, tag=f"A{bi}", name=f"A{bi}")
                        nc.sync.dma_start(
                            out=a, in_=dram[f"s{si}_A"].ap()[st : st + sz, :]
                        )
                        A_t.append(a)
                        cwt = res.tile([sz, DCONV], f32, tag=f"cw{bi}", name=f"cw{bi}")
                        nc.sync.dma_start(
                            out=cwt, in_=dram[f"s{si}_cw"].ap()[st : st + sz, :]
                        )
                        cw_t.append(cwt)
                        w = res.tile(
                            [sz, 2 * DSTATE], bf16, tag=f"bcw{bi}", name=f"bcw{bi}"
                        )
                        nc.sync.dma_start(
                            out=w, in_=dram[f"s{si}_bcwT"].ap()[st : st + sz, :]
                        )
                        bcw_t.append(w)

                    Wt_t = []
                    for ci, (cst, csz) in enumerate(c_p):
                        w = res.tile([csz, m], bf16, tag=f"Wt{ci}", name=f"Wt{ci}")
                        nc.sync.dma_start(
                            out=w, in_=dram[f"s{si}_Wt"].ap()[cst : cst + csz, :]
                        )
                        Wt_t.append(w)

                    # cross-chunk state
                    halo_t, hcar_t, mxa_t, sma_t = [], [], [], []
                    for bi, (st, sz) in enumerate(d_p):
                        ht = res.tile(
                            [sz, DCONV - 1], f32, tag=f"halo{bi}", name=f"halo{bi}"
                        )
                        nc.vector.memset(ht, 0.0)
                        halo_t.append(ht)
                        hc = res.tile(
                            [sz, DSTATE], f32, tag=f"hcar{bi}", name=f"hcar{bi}"
                        )
                        nc.vector.memset(hc, 0.0)
                        hcar_t.append(hc)
                    if last:
                        for mi, (mst, msz) in enumerate(m_p):
                            mx = res.tile([msz, 1], f32, tag=f"mxa{mi}", name=f"mxa{mi}")
                            mxa_t.append(mx)
                            sm = res.tile([msz, 1], f32, tag=f"sma{mi}", name=f"sma{mi}")
                            sma_t.append(sm)

                    # stage output (full width, persists into next stage)
                    o_t = []
                    if not last:
                        for mi, (mst, msz) in enumerate(m_p):
                            o = acts.tile(
                                [msz, N_], bf16, tag=f"o{si}_{mi}", name=f"o{si}_{mi}"
                            )
                            o_t.append(o)

                    for ch in range(nch):
                        cb = ch * CN

                        def gsl(f):  # global column slice of 512-tile f
                            return slice(cb + f * NF, cb + (f + 1) * NF)

                        def lsl(f):  # chunk-local slice
                            return slice(f * NF, (f + 1) * NF)

                        # ---- linear + BN + relu ----
                        r_t = []
                        for mi, (mst, msz) in enumerate(m_p):
                            r = res.tile([msz, CN], bf16, tag=f"r{mi}", name=f"r{mi}", bufs=(2 if nch > 1 else 1))
                            r_t.append(r)
                            for f in range(NFC):
                                pl = psmm.tile([msz, NF], f32, tag="mm")
                                for ci, (cst, csz) in enumerate(c_p):
                                    nc.tensor.matmul(
                                        out=pl,
                                        lhsT=Wt_t[ci][:, mst : mst + msz],
                                        rhs=x_tiles[ci][:, gsl(f)],
                                        start=(ci == 0),
                                        stop=(ci == len(c_p) - 1),
                                    )
                                nc.scalar.activation(
                                    out=r[:, lsl(f)],
                                    in_=pl,
                                    func=Act.Relu,
                                    bias=b_v[mi],
                                    scale=1.0,
                                )

                        # ---- in_proj, conv, silu, D-init, B/C ----
                        delta_t, y_t = [], []
                        bc_ps = ps1.tile([2 * DSTATE, CN], f32, tag="big")
                        for bi, (dst, dsz) in enumerate(d_p):
                            xcp = tmp.tile(
                                [dsz, DCONV - 1 + CN], bf16, tag="xcpad"
                            )
                            nc.vector.tensor_copy(
                                out=xcp[:, : DCONV - 1], in_=halo_t[bi]
                            )
                            for f in range(NFC):
                                pxc = psmm.tile([dsz, NF], f32, tag="mm")
                                for mi, (mst, msz) in enumerate(m_p):
                                    wl = wpool.tile(
                                        [msz, dsz], bf16, tag=f"wxc{mi}", name="wl"
                                    )
                                    nc.sync.dma_start(
                                        out=wl,
                                        in_=dram[f"s{si}_inwT"].ap()[
                                            mst : mst + msz, dst : dst + dsz
                                        ],
                                    )
                                    nc.tensor.matmul(
                                        out=pxc,
                                        lhsT=wl,
                                        rhs=r_t[mi][:, lsl(f)],
                                        start=(mi == 0),
                                        stop=(mi == len(m_p) - 1),
                                    )
                                nc.scalar.activation(
                                    out=xcp[
                                        :,
                                        DCONV - 1 + f * NF : DCONV - 1 + (f + 1) * NF,
                                    ],
                                    in_=pxc,
                                    func=Act.Identity,
                                    bias=inb_v[dst // 128],
                                    scale=1.0,
                                )
                            # save halo for next chunk (last 3 columns)
                            if ch + 1 < nch:
                                nc.vector.tensor_copy(
                                    out=halo_t[bi], in_=xcp[:, CN : CN + DCONV - 1]
                                )
                            dlt = res.tile(
                                [dsz, CN], bf16, tag=f"delta{bi}", name=f"delta{bi}",
                                bufs=(2 if nch > 1 else 1),
                            )
                            delta_t.append(dlt)
                            zst = di + dst
                            for f in range(NFC):
                                pz = psmm.tile([dsz, NF], f32, tag="mm")
                                for mi, (mst, msz) in enumerate(m_p):
                                    wl = wpool.tile(
                                        [msz, dsz], bf16, tag=f"wz{mi}", name="wl"
                                    )
                                    nc.sync.dma_start(
                                        out=wl,
                                        in_=dram[f"s{si}_inwT"].ap()[
                                            mst : mst + msz, zst : zst + dsz
                                        ],
                                    )
                                    nc.tensor.matmul(
                                        out=pz,
                                        lhsT=wl,
                                        rhs=r_t[mi][:, lsl(f)],
                                        start=(mi == 0),
                                        stop=(mi == len(m_p) - 1),
                                    )
                                nc.scalar.activation(
                                    out=dlt[:, lsl(f)],
                                    in_=pz,
                                    func=Act.Sigmoid,
                                    bias=inb_v[zst // 128],
                                    scale=1.0,
                                )
                            acc = tmp.tile([dsz, CN], f32, tag="convacc", bufs=2)
                            nc.vector.tensor_scalar_mul(
                                out=acc, in0=xcp[:, 0:CN], scalar1=cw_t[bi][:, 0:1]
                            )
                            for k in range(1, DCONV):
                                nc.vector.scalar_tensor_tensor(
                                    out=acc,
                                    in0=xcp[:, k : k + CN],
                                    scalar=cw_t[bi][:, k : k + 1],
                                    in1=acc,
                                    op0=Alu.mult,
                                    op1=Alu.add,
                                )
                            nc.vector.tensor_scalar_add(
                                out=acc, in0=acc, scalar1=cb_v[bi]
                            )
                            sg = tmp.tile([dsz, CN], bf16, tag="sg", bufs=2)
                            nc.scalar.activation(
                                out=sg, in_=acc, func=Act.Sigmoid, bias=0.0, scale=1.0
                            )
                            xt = tmp.tile([dsz, CN], bf16, tag="xt", bufs=2)
                            nc.vector.tensor_mul(xt, acc, sg)
                            y = res.tile([dsz, CN], bf16, tag=f"y{bi}", name=f"y{bi}", bufs=(2 if nch > 1 else 1))
                            y_t.append(y)
                            nc.vector.tensor_scalar_mul(out=y, in0=xt, scalar1=D_v[bi])
                            for f in range(NFC):
                                nc.tensor.matmul(
                                    out=bc_ps[:, lsl(f)],
                                    lhsT=bcw_t[bi],
                                    rhs=xt[:, lsl(f)],
                                    start=(bi == 0),
                                    stop=(bi == len(d_p) - 1),
                                )
                        bc_sb = res.tile([2 * DSTATE, CN], bf16, tag="bc_sb", bufs=(2 if nch > 1 else 1))
                        nc.scalar.copy(out=bc_sb, in_=bc_ps)

                        # ---- scan ----
                        for s in range(DSTATE):
                            brow = tmp.tile([1, CN], bf16, tag="brow", bufs=2)
                            nc.sync.dma_start(out=brow, in_=bc_sb[s : s + 1, :])
                            crow = tmp.tile([1, CN], bf16, tag="crow", bufs=2)
                            nc.sync.dma_start(
                                out=crow, in_=bc_sb[DSTATE + s : DSTATE + s + 1, :]
                            )
                            brep = tmp.tile([128, CN], bf16, tag="brep", bufs=2)
                            crep = tmp.tile([128, CN], bf16, tag="crep", bufs=2)
                            for f in range(NFC):
                                brep_ps = psmm.tile([128, NF], f32, tag="mm")
                                nc.tensor.matmul(
                                    out=brep_ps,
                                    lhsT=ones_bf,
                                    rhs=brow[:, lsl(f)],
                                    start=True,
                                    stop=True,
                                )
                                nc.scalar.copy(out=brep[:, lsl(f)], in_=brep_ps)
                                crep_ps = psmm.tile([128, NF], f32, tag="mm")
                                nc.tensor.matmul(
                                    out=crep_ps,
                                    lhsT=ones_bf,
                                    rhs=crow[:, lsl(f)],
                                    start=True,
                                    stop=True,
                                )
                                nc.scalar.copy(out=crep[:, lsl(f)], in_=crep_ps)

                            for bi, (dst, dsz) in enumerate(d_p):
                                h = tmp.tile([dsz, NF], bf16, tag="h", name="h")
                                for f in range(NFC):
                                    u = tmp.tile([dsz, NF], bf16, tag="u", bufs=4)
                                    nc.vector.tensor_mul(
                                        u, delta_t[bi][:, lsl(f)], brep[:dsz, lsl(f)]
                                    )
                                    init = (
                                        hcar_t[bi][:, s : s + 1]
                                        if f == 0
                                        else h[:, NF - 1 : NF]
                                    )
                                    nc.vector.tensor_tensor_scan(
                                        out=h,
                                        data0=bcast_free(A_t[bi][:, s : s + 1], NF),
                                        data1=u,
                                        initial=init,
                                        op0=Alu.mult,
                                        op1=Alu.add,
                                    )
                                    hc = tmp.tile([dsz, NF], bf16, tag="hc", bufs=4)
                                    nc.vector.tensor_mul(hc, h, crep[:dsz, lsl(f)])
                                    nc.vector.tensor_add(
                                        y_t[bi][:, lsl(f)], y_t[bi][:, lsl(f)], hc
                                    )
                                if ch + 1 < nch:
                                    nc.vector.tensor_copy(
                                        out=hcar_t[bi][:, s : s + 1],
                                        in_=h[:, NF - 1 : NF],
                                    )

                        # ---- out_proj + ob + residual (written in-place into r) ----
                        op_t = r_t
                        for mi, (mst, msz) in enumerate(m_p):
                            for f in range(NFC):
                                po = psmm.tile([msz, NF], f32, tag="mm")
                                for bi, (dst, dsz) in enumerate(d_p):
                                    wl = wpool.tile(
                                        [dsz, msz], bf16, tag=f"wo{bi % 4}", name="wl"
                                    )
                                    nc.sync.dma_start(
                                        out=wl,
                                        in_=dram[f"s{si}_owT"].ap()[
                                            dst : dst + dsz, mst : mst + msz
                                        ],
                                    )
                                    nc.tensor.matmul(
                                        out=po,
                                        lhsT=wl,
                                        rhs=y_t[bi][:, lsl(f)],
                                        start=(bi == 0),
                                        stop=(bi == len(d_p) - 1),
                                    )
                                nc.vector.scalar_tensor_tensor(
                                    out=op_t[mi][:, lsl(f)],
                                    in0=po,
                                    scalar=ob_v[mi],
                                    in1=r_t[mi][:, lsl(f)],
                                    op0=Alu.add,
                                    op1=Alu.add,
                                )

                        # ---- LayerNorm over channels ----
                        mu_ps = ps1.tile([1, CN], f32, tag="big")
                        for mi, (mst, msz) in enumerate(m_p):
                            for f in range(NFC):
                                nc.tensor.matmul(
                                    out=mu_ps[:, lsl(f)],
                                    lhsT=ones_col[:msz, :],
                                    rhs=op_t[mi][:, lsl(f)],
                                    start=(mi == 0),
                                    stop=(mi == len(m_p) - 1),
                                )
                        mu_sb = tmp.tile([1, CN], f32, tag="stat_sb", bufs=4)
                        nc.scalar.mul(out=mu_sb, in_=mu_ps, mul=1.0 / m)
                        for f in range(NFC):
                            murep = psmm.tile([128, NF], f32, tag="mm")
                            nc.tensor.matmul(
                                out=murep,
                                lhsT=ones_f32,
                                rhs=mu_sb[:, lsl(f)],
                                start=True,
                                stop=True,
                            )
                            for mi, (mst, msz) in enumerate(m_p):
                                nc.vector.tensor_sub(
                                    op_t[mi][:, lsl(f)],
                                    op_t[mi][:, lsl(f)],
                                    murep[:msz, :],
                                )
                        var_ps = ps1.tile([1, CN], f32, tag="big")
                        for mi, (mst, msz) in enumerate(m_p):
                            sq = tmp.tile([msz, CN], bf16, tag="sq")
                            nc.scalar.square(out=sq, in_=op_t[mi])
                            for f in range(NFC):
                                nc.tensor.matmul(
                                    out=var_ps[:, lsl(f)],
                                    lhsT=ones_col[:msz, :],
                                    rhs=sq[:, lsl(f)],
                                    start=(mi == 0),
                                    stop=(mi == len(m_p) - 1),
                                )
                        # rstd = 1/sqrt(var/m + eps), computed as
                        # 2^-32 / sqrt((var/m + eps) * 2^-64) so the ACT sqrt
                        # stays in its valid range even for huge variances;
                        # inf clamps to 2^110 (rstd ~ 0) and NaN columns are
                        # scrubbed for the sqrt then re-injected.
                        t_v = tmp.tile([1, CN], f32, tag="stat_sb", bufs=4)
                        nc.vector.tensor_scalar(
                            out=t_v,
                            in0=var_ps,
                            scalar1=(2.0 ** -64) / m,
                            scalar2=EPS * (2.0 ** -64),
                            op0=Alu.mult,
                            op1=Alu.add,
                        )
                        mnan = tmp.tile([1, CN], f32, tag="stat_sb", bufs=4)
                        nc.vector.tensor_tensor(
                            out=mnan, in0=t_v, in1=t_v, op=Alu.not_equal
                        )
                        mnan8 = tmp.tile(
                            [1, CN], mybir.dt.uint8, tag="mnan8", bufs=2
                        )
                        nc.vector.tensor_tensor(
                            out=mnan8, in0=t_v, in1=t_v, op=Alu.not_equal
                        )
                        t_c = tmp.tile([1, CN], f32, tag="stat_sb", bufs=4)
                        nc.vector.tensor_scalar_min(
                            out=t_c, in0=t_v, scalar1=2.0 ** 110
                        )
                        nc.vector.copy_predicated(
                            out=t_c, mask=mnan8, data=bcast_free(big_t, CN)
                        )
                        std_sb = tmp.tile([1, CN], f32, tag="stat_sb", bufs=4)
                        nc.scalar.activation(
                            out=std_sb, in_=t_c, func=Act.Sqrt, bias=0.0, scale=1.0
                        )
                        r1_sb = tmp.tile([1, CN], f32, tag="stat_sb", bufs=4)
                        nc.vector.reciprocal(out=r1_sb, in_=std_sb)
                        # nanadd: NaN where var was NaN, else 0
                        nc.vector.tensor_scalar_mul(
                            out=mnan, in0=mnan, scalar1=3.0e38
                        )
                        nc.vector.tensor_mul(mnan, mnan, mnan)
                        nc.vector.tensor_scalar_mul(out=mnan, in0=mnan, scalar1=0.0)
                        rstd_sb = tmp.tile([1, CN], f32, tag="stat_sb", bufs=4)
                        nc.vector.scalar_tensor_tensor(
                            out=rstd_sb,
                            in0=r1_sb,
                            scalar=2.0 ** -32,
                            in1=mnan,
                            op0=Alu.mult,
                            op1=Alu.add,
                        )

                        for f in range(NFC):
                            rstdrep = psmm.tile([128, NF], f32, tag="mm")
                            nc.tensor.matmul(
                                out=rstdrep,
                                lhsT=ones_f32,
                                rhs=rstd_sb[:, lsl(f)],
                                start=True,
                                stop=True,
                            )
                            for mi, (mst, msz) in enumerate(m_p):
                                if last:
                                    od = tmp.tile(
                                        [msz, NF], bf16, tag="ofin", bufs=3, name="od"
                                    )
                                else:
                                    od = o_t[mi][:, gsl(f)]
                                nc.vector.scalar_tensor_tensor(
                                    out=od,
                                    in0=op_t[mi][:, lsl(f)],
                                    scalar=lng_v[mi],
                                    in1=rstdrep[:msz, :],
                                    op0=Alu.mult,
                                    op1=Alu.mult,
                                )
                                nc.vector.tensor_scalar_add(
                                    out=od, in0=od, scalar1=lnb_v[mi]
                                )
                                if last:
                                    # accumulate max and sum over this 512-col piece
                                    mx = tmp.tile([msz, 1], f32, tag="mx", bufs=3)
                                    nc.vector.tensor_reduce(
                                        out=mx,
                                        in_=od,
                                        axis=mybir.AxisListType.X,
                                        op=Alu.max,
                                    )
                                    sm = tmp.tile([msz, 1], f32, tag="sm", bufs=3)
                                    nc.vector.tensor_reduce(
                                        out=sm,
                                        in_=od,
                                        axis=mybir.AxisListType.X,
                                        op=Alu.add,
                                    )
                                    if ch == 0 and f == 0:
                                        nc.vector.tensor_copy(out=mxa_t[mi], in_=mx)
                                        nc.vector.tensor_copy(out=sma_t[mi], in_=sm)
                                    else:
                                        nc.vector.tensor_max(
                                            mxa_t[mi], mxa_t[mi], mx
                                        )
                                        nc.vector.tensor_add(
                                            sma_t[mi], sma_t[mi], sm
                                        )
                                if DEBUG_STAGE_OUTS:
                                    of = tmp.tile(
                                        [msz, NF], f32, tag="dbg_cp", name="of"
                                    )
                                    nc.vector.tensor_copy(out=of, in_=od)
                                    nc.sync.dma_start(
                                        out=dbg[si].ap()[mst : mst + msz, gsl(f)],
                                        in_=of,
                                    )

                    if last:
                        for mi, (mst, msz) in enumerate(m_p):
                            fx = tmp.tile([msz, 1], f32, tag="fx", bufs=2)
                            nc.vector.tensor_scalar_mul(
                                out=fx, in0=sma_t[mi], scalar1=0.0
                            )
                            rs = tmp.tile([msz, 1], f32, tag="rs", bufs=2)
                            nc.vector.tensor_add(rs, mxa_t[mi], fx)
                            nc.sync.dma_start(
                                out=dram["out"].ap()[mst : mst + msz, :], in_=rs
                            )
                    else:
                        x_tiles = o_t
    if compile:
        nc.compile()
    return nc


_CACHE = {}


def kernel(points, params):
    """points: [8, 2048, 3] fp32; params: list of 3 stage dicts. Returns [8, 1024] fp32."""
    from concourse.bass_utils import run_bass_kernel_spmd

    pts = np.asarray(points, np.float32)
    arrs = prep_params(params)

    if "prog" not in _CACHE:
        _CACHE["prog"] = build_program(arrs)
    nc = _CACHE["prog"]

    in_maps = []
    for i in range(NCORES):
        im = dict(arrs)
        im["xin"] = np.ascontiguousarray(pts[i].T).astype(np.float32)
        in_maps.append(im)

    res = run_bass_kernel_spmd(nc, in_maps, core_ids=list(range(NCORES)))
    out = np.stack([r["out"][:, 0] for r in res.results]).astype(np.float32)
    return out
